# revision 22
# baseline (speedup 1.0000x reference)
"""Causal self-attention kernel for Trainium2, 8 NeuronCores.

Sharding: DP4 x TP2. Core c = 2*b + g handles batch b (2048 tokens) and
head-group g (8 of 16 heads). Per core:
  - x arrives pre-cast to bf16 and is transposed by the DMA xbar
    (d_model onto partitions), no PE involvement,
  - QKV matmuls in bf16: Q,K dim-major ([head_dim, tokens]), V token-major
    padded to 128 columns with a ones column (softmax denominator for free),
  - attention per head pair: scores^T = K_h^T-tile @ Q_h in [k, q] layout
    with both heads' QK matmuls in different PE row groups (concurrent),
    one wide exp on ACT (1/sqrt(64) folded into its scale) into bf16 probs,
    causal handling by skipping fully-masked tiles, memset on fully-masked
    column ranges and a 0/1 mask multiply on the 128-wide diagonal band,
  - normalization via reciprocal_approx_fast + gpsimd partition_broadcast,
  - fp32r projection with the w_proj row shard (token-major output),
  - chunked pairwise AllReduce (cores 2b, 2b+1) overlapped with later tiles.

QKV work for token tile n+1 is emitted interleaved with attention for tile n
so the PE always has independent matmuls while ACT drains the exps.

Everything (shapes, sharding) is hardcoded for
x: [4, 2048, 1024], w_qkv: [1024, 3072], w_proj: [1024, 1024], f32.
"""

import ml_dtypes
import numpy as np

import concourse.bacc as bacc
import concourse.mybir as mybir
import concourse.tile as tile
from concourse.tile import add_dep_helper
from concourse.bass_utils import run_bass_kernel_spmd

F32 = mybir.dt.float32
F32R = mybir.dt.float32r
BF16 = mybir.dt.bfloat16

S = 2048  # tokens per core (one batch element)
D = 1024  # d_model
HL = 8  # heads per core (local)
HD = 64  # head dim
GD = HL * HD  # 512, head-group dim
NQT = S // 512  # 4 q-tiles of 512
NDM = D // 128  # 8 d_model chunks
NTOK = S // 128  # 16 token tiles of 128

_NC_CACHE = {}


def _qkv_units(nc, P, n):
    """QKV matmul chains for token tile n, as separately emittable units."""
    units = []

    def qk_chain(m):
        def emit():
            ps = P.b1_ps.tile([128, 512], F32, tag="b1", name="qkps")
            for k in range(NDM):
                nc.tensor.matmul(
                    ps,
                    P.w_sb[:, k, m * 128 : (m + 1) * 128],
                    P.xT[:, k, n * 512 : (n + 1) * 512],
                    start=(k == 0),
                    stop=(k == NDM - 1),
                )
            nc.vector.tensor_copy(
                out=P.qkT[:, m, n * 512 : (n + 1) * 512], in_=ps
            )

        return emit

    def v_chain(t4):
        def emit():
            t = n * 4 + t4
            ps = P.b1_ps.tile([128, 512], F32, tag="b1", name="vps")
            for k in range(NDM):
                nc.tensor.matmul(
                    ps,
                    P.xT[:, k, t * 128 : (t + 1) * 128],
                    P.w_sb[:, k, 2 * GD : 3 * GD],
                    start=(k == 0),
                    stop=(k == NDM - 1),
                )
            nc.vector.tensor_copy(
                out=P.v_sb[:, t, :, 0:HD],
                in_=ps.rearrange("p (h d) -> p h d", h=HL),
            )

        return emit

    for m in range(2 * GD // 128):
        units.append(qk_chain(m))
    for t4 in range(4):
        units.append(v_chain(t4))
    return units


def _attn_units(nc, P, j):
    """Attention units for q-tile j: per head pair, c-groups + epilogue."""
    units = []
    for hp in range(HL // 2):
        yps = {}

        def alloc(hp=hp, yps=yps):
            for hi in range(2):
                yps[hi] = P.y_ps.tile(
                    [128, 512], F32, tag=f"yps{hi}", name=f"yps{hi}", bufs=1
                )

        units.append(alloc)

        def cgroup(c, hp=hp, yps=yps):
            def emit():
                d = c - 4 * j  # >= 0 on the diagonal band
                off = max(d, 0) * 128  # columns below off are fully masked
                sps2 = P.attn_ps.tile(
                    [128, 2, 512], F32, tag="sps2", name="sps2"
                )
                for hi in range(2):
                    h = 2 * hp + hi
                    po = (h % 2) * 64
                    nc.tensor.matmul(
                        sps2[:, hi, off:512],
                        P.qkT[po : po + 64, 4 + h // 2, c * 128 : (c + 1) * 128],
                        P.qkT[po : po + 64, h // 2, j * 512 + off : (j + 1) * 512],
                        start=True,
                        stop=True,
                    )
                probs2 = P.probs_p.tile(
                    [128, 2, 512], BF16, tag="probs", name="probs"
                )
                if off:
                    nc.vector.memset(probs2[:, :, 0:off], 0.0)
                nc.scalar.activation(
                    out=probs2[:, :, off:512],
                    in_=sps2[:, :, off:512],
                    func=mybir.ActivationFunctionType.Exp,
                    scale=0.125,
                )
                if d >= 0:
                    for hi in range(2):
                        nc.vector.tensor_mul(
                            probs2[:, hi, off : off + 128],
                            probs2[:, hi, off : off + 128],
                            P.mask_sb,
                        )
                for hi in range(2):
                    h = 2 * hp + hi
                    nc.tensor.matmul(
                        yps[hi],
                        P.v_sb[:, c, h, :],
                        probs2[:, hi, :],
                        start=(c == 0),
                        stop=(c == 4 * j + 3),
                    )

            return emit

        for c in range(4 * j + 4):
            units.append(cgroup(c))

        def epilogue(hp=hp, yps=yps):
            # ones-row out of PSUM, fast reciprocal, partition broadcast,
            # scale y into dim-major yT
            for hi in range(2):
                h = 2 * hp + hi
                po = (h % 2) * 64
                den = P.den_p.tile([1, 512], F32, tag="den", name="den")
                nc.scalar.activation(
                    out=den,
                    in_=yps[hi][HD : HD + 1, :],
                    func=mybir.ActivationFunctionType.Copy,
                )
                nc.vector.reciprocal_approx_fast(out=den, in_=den)
                denb = P.den_p.tile([HD, 512], F32, tag="denb", name="denb")
                nc.gpsimd.partition_broadcast(denb, den)
                nc.vector.tensor_mul(
                    P.yT[po : po + 64, h // 2, j * 512 : (j + 1) * 512],
                    yps[hi][0:HD, :],
                    denb,
                )

        units.append(epilogue)
    return units


def _proj_chunk(nc, P, j):
    """Projection for the 4 token tiles of q-tile j (token-major output)."""
    for mt in range(4 * j, 4 * j + 4):
        osb = P.out_p.tile([128, D], F32, tag="osb", name="osb")
        for nh in range(2):
            ps = P.b1_ps.tile([128, 512], F32, tag="b1", name="ops")
            for kk in range(GD // 128):
                nc.tensor.matmul(
                    ps,
                    P.yT[:, kk, mt * 128 : (mt + 1) * 128],
                    P.wp_sb[:, kk, nh * 512 : (nh + 1) * 512],
                    start=(kk == 0),
                    stop=(kk == GD // 128 - 1),
                )
            nc.vector.tensor_copy(out=osb[:, nh * 512 : (nh + 1) * 512], in_=ps)
        nc.sync.dma_start(out=P.cc_in[mt * 128 : (mt + 1) * 128, :], in_=osb)


def _ar_chunk(nc, P, j):
    """AllReduce + final output DMA for q-tile j's 512 token rows."""
    lo, hi = j * 512, (j + 1) * 512
    nc.gpsimd.collective_compute(
        "AllReduce",
        mybir.AluOpType.add,
        replica_groups=[[0, 1], [2, 3], [4, 5], [6, 7]],
        ins=[P.cc_in[lo:hi, :].opt()],
        outs=[P.cc_out[lo:hi, :].opt()],
    )
    for mt in range(4 * j, 4 * j + 4):
        nc.sync.dma_start(
            out=P.out[mt * 128 : (mt + 1) * 128, :],
            in_=P.cc_out[mt * 128 : (mt + 1) * 128, :],
        )


class _Ctx:
    pass


def _build_nc():
    nc = bacc.Bacc(None, num_devices=8)
    P = _Ctx()

    xb16 = nc.dram_tensor("xb16", [S, D], BF16, kind="ExternalInput").ap()
    wqkv = nc.dram_tensor("wqkv", [D, 3 * GD], BF16, kind="ExternalInput").ap()
    wproj = nc.dram_tensor("wproj", [GD, D], F32, kind="ExternalInput").ap()
    masks = nc.dram_tensor("masks", [128, 128], BF16, kind="ExternalInput").ap()
    P.out = nc.dram_tensor("out", [S, D], F32, kind="ExternalOutput").ap()

    with tile.TileContext(nc) as tc:
        with (
            tc.tile_pool(name="const", bufs=1) as const,
            tc.tile_pool(name="w_p", bufs=1) as w_p,
            tc.tile_pool(name="big_p", bufs=1) as big_p,
            tc.tile_pool(name="probs_p", bufs=6) as probs_p,
            tc.tile_pool(name="den_p", bufs=2) as den_p,
            tc.tile_pool(name="out_p", bufs=2) as out_p,
            tc.tile_pool(name="b1_ps", bufs=2, space="PSUM") as b1_ps,
            tc.tile_pool(name="attn_ps", bufs=2, space="PSUM") as attn_ps,
            tc.tile_pool(name="y_ps", bufs=1, space="PSUM") as y_ps,
            tc.tile_pool(name="dram", bufs=1, space="DRAM") as dram,
        ):
            P.probs_p, P.den_p, P.out_p = probs_p, den_p, out_p
            P.b1_ps, P.attn_ps, P.y_ps = b1_ps, attn_ps, y_ps

            # DMA xbar transpose first: xT[p, e, t] = x[t, e*128 + p].
            # The xbar transpose silently corrupts data when plain DMAs run
            # concurrently, so every other startup DMA gets an explicit
            # dependency edge on all transposes.
            P.xT = big_p.tile([128, NDM, S], BF16, name="xT")
            tr_insts = []
            for t in range(NTOK):
                tr_insts.append(
                    nc.sync.dma_start_transpose(
                        out=P.xT[:, :, t * 128 : (t + 1) * 128],
                        in_=xb16[t * 128 : (t + 1) * 128, :],
                    )
                )
            plain = []
            P.mask_sb = const.tile([128, 128], BF16, name="mask_sb")
            plain.append(nc.sync.dma_start(out=P.mask_sb, in_=masks))

            P.w_sb = w_p.tile([128, NDM, 3 * GD], BF16, name="w_sb")
            for k in range(NDM):
                plain.append(
                    nc.sync.dma_start(
                        out=P.w_sb[:, k, :], in_=wqkv[k * 128 : (k + 1) * 128, :]
                    )
                )
            P.wp_sb = w_p.tile([128, GD // 128, D], F32R, name="wp_sb")
            for kk in range(GD // 128):
                plain.append(
                    nc.sync.dma_start(
                        out=P.wp_sb[:, kk, :],
                        in_=wproj[kk * 128 : (kk + 1) * 128, :].bitcast(F32R),
                    )
                )
            for p_ in plain:
                for ti in tr_insts:
                    add_dep_helper(
                        p_.ins, ti.ins, sync=True,
                        reason="xbar transpose isolation",
                    )
            P.qkT = big_p.tile([128, 2 * GD // 128, S], BF16, name="qkT")
            P.v_sb = big_p.tile([128, NTOK, HL, 128], BF16, name="v_sb")
            nc.vector.memset(P.v_sb[:, :, :, HD:128], 0.0)
            nc.vector.memset(P.v_sb[:, :, :, HD : HD + 1], 1.0)
            P.yT = big_p.tile([128, GD // 128, S], F32R, name="yT")

            P.cc_in = dram.tile([S, D], F32, name="cc_in")
            P.cc_out = dram.tile([S, D], F32, name="cc_out")

            ZIPPER = False
            for u in _qkv_units(nc, P, 0):
                u()
            for n in range(NQT):
                a_units = _attn_units(nc, P, n)
                q_units = _qkv_units(nc, P, n + 1) if n + 1 < NQT else []
                qi = 0
                for i, u in enumerate(a_units):
                    u()
                    while ZIPPER and qi < len(q_units) and qi * len(
                        a_units
                    ) < (i + 1) * len(q_units):
                        q_units[qi]()
                        qi += 1
                for u in q_units[qi:]:
                    u()
                _proj_chunk(nc, P, n)
                _ar_chunk(nc, P, n)

    nc.compile()
    return nc


def _host_consts():
    ki = np.arange(128)[:, None]
    qj = np.arange(128)[None, :]
    masks = (qj >= ki).astype(ml_dtypes.bfloat16)  # [128, 128] diagonal band
    return masks


def _in_maps(x, w_qkv, w_proj):
    masks = _host_consts()
    maps = []
    for c in range(8):
        b, g = c // 2, c % 2
        wq = w_qkv[:, g * GD : (g + 1) * GD]
        wk = w_qkv[:, D + g * GD : D + (g + 1) * GD]
        wv = w_qkv[:, 2 * D + g * GD : 2 * D + (g + 1) * GD]
        maps.append(
            {
                "xb16": np.ascontiguousarray(x[b]).astype(ml_dtypes.bfloat16),
                "wqkv": np.ascontiguousarray(
                    np.concatenate([wq, wk, wv], axis=1)
                ).astype(ml_dtypes.bfloat16),
                "wproj": np.ascontiguousarray(w_proj[g * GD : (g + 1) * GD, :]),
                "masks": masks,
            }
        )
    return maps


def kernel(x, w_qkv, w_proj):
    x = np.ascontiguousarray(x, dtype=np.float32)
    w_qkv = np.ascontiguousarray(w_qkv, dtype=np.float32)
    w_proj = np.ascontiguousarray(w_proj, dtype=np.float32)
    if "nc" not in _NC_CACHE:
        _NC_CACHE["nc"] = _build_nc()
    nc = _NC_CACHE["nc"]
    r = run_bass_kernel_spmd(nc, _in_maps(x, w_qkv, w_proj), list(range(8)))
    return np.stack([r.results[2 * b]["out"] for b in range(4)], axis=0)
